# revision 12
# baseline (speedup 1.0000x reference)
"""Multi-head attention (B=4, S=2048, D=1024, H=16) on 8 TRN2 NeuronCores.

Sharding: core c -> (batch b = c//2, head-group g = c%2). Each core computes
8 heads for one batch: QKV projections restricted to its 512 output dims,
attention for its heads, and a partial output projection over its 512
contraction dims of W_o. Host sums the two partial outputs per batch.

v7 redesign vs v6:
  * q-chunks shrunk to 512 so PSUM fits qk(4) + pv(2) + fo(2) banks and the
    output projection interleaves with attention per q-chunk instead of
    running as a serial epilogue.
  * softmax normalize (aT = otX * 1/Z) and the ones-column memset moved to
    the otherwise-idle GPSIMD engine; psum drains alternate DVE/ScalarE.
  * exp split ScalarE/DVE parametrized (se_num/16 on ScalarE ACT-Exp, rest
    on the DVE Schraudolph bit-trick exp).
  * QK head pairs stay row-packed (64-row lhsT at partitions 0/64) for
    tile_position concurrency on hardware.
  * PV stays V-stationary ([128k, 65] incl. ones column accumulating the
    softmax denominator Z in psum row 64); output lands as O^T [64d, q].

Device layout (per core, matmul operands bf16, PSUM fp32):
  inputs  xqT/xkT/xvT [1024, S]   (X^T: feature dim on partitions)
          wqT/wkT/wvT [1024, 512] (W.T slices; 1/sqrt(dk) folded into wqT)
          woT         [512, 1024] (W_o slice transposed)
  output  outT        [1024, S]   (partial final^T, fp32)
"""

import numpy as np
import ml_dtypes

B = 4
S = 2048
D = 1024
H_LOCAL = 8          # heads per core
DK = 64
G = H_LOCAL * DK     # 512 output dims per core
N_CORES = 8

_BUILD_CACHE = {}
_BUILD_VERSION = 16  # bump on any device-program change: busts the neuronxcc
                     # cache, whose module hash ignores custom_call contents

bf16 = ml_dtypes.bfloat16

# Schraudolph exp in bf16: bits = int16(x * 128/ln2 + (127*128 - 7.3)),
# bitcast to bf16.  ~+-3% rel err; the mean multiplicative bias cancels in
# softmax (shared by numerator and denominator Z).
SCH_A = 184.66502678663007
SCH_B = 16248.7


def _build(s=S, debug=False, stage=5, delay_us=0, reps=1, dve_exp=True,
           qk512=True, se_num=9):
    """stage: 1=DMA only, 2=+QKV proj, 3=+QK/exp, 35=+PV, 4=+normalize,
    5=full.  reps wraps the body in a For_i hardware loop (timing builds);
    the body is idempotent so outputs are unchanged.
    se_num: out of 16 exp tiles, how many go to ScalarE (rest DVE)."""
    import contextlib
    import concourse.tile as tile
    from concourse import bacc, mybir

    f32 = mybir.dt.float32
    b16 = mybir.dt.bfloat16
    i16 = mybir.dt.int16

    pv_only = (stage == 35)
    if pv_only:
        stage = 4

    assert s % 512 == 0
    NKT = s // 128          # k-position tiles per head
    QW = 512                # q-chunk width for the attention pipeline
    NQH = s // QW           # q chunks
    HALF = 512              # projection psum tile width
    NH = s // HALF

    nc = bacc.Bacc("TRN2", target_bir_lowering=False, debug=False,
                   num_devices=N_CORES)

    xqT = nc.dram_tensor("xqT", [D, s], b16, kind="ExternalInput")
    xkT = nc.dram_tensor("xkT", [D, s], b16, kind="ExternalInput")
    xvT = nc.dram_tensor("xvT", [D, s], b16, kind="ExternalInput")
    wqT = nc.dram_tensor("wqT", [D, G], b16, kind="ExternalInput")
    wkT = nc.dram_tensor("wkT", [D, G], b16, kind="ExternalInput")
    wvT = nc.dram_tensor("wvT", [D, G], b16, kind="ExternalInput")
    woT = nc.dram_tensor("woT", [G, D], b16, kind="ExternalInput")
    nc.dram_tensor("vtag", [stage + 8 * reps + (64 if dve_exp else 0)
                            + (90 if pv_only else 0) + (500 if qk512 else 0)
                            + 7 * se_num,
                            _BUILD_VERSION + delay_us], f32,
                   kind="ExternalInput")
    outT = nc.dram_tensor("outT", [D, s], f32, kind="ExternalOutput")
    if debug:
        qTd = nc.dram_tensor("qTd", [G, s], b16, kind="ExternalOutput")
        kTd = nc.dram_tensor("kTd", [G, s], b16, kind="ExternalOutput")
        vd = nc.dram_tensor("vd", [s, H_LOCAL * (DK + 1)], b16,
                            kind="ExternalOutput")
        aTd = nc.dram_tensor("aTd", [G, s], b16, kind="ExternalOutput")

    QKB = 2   # psum bufs for the shared "qk" tag (2 banks each)
    Exp = mybir.ActivationFunctionType.Exp
    MULT = mybir.AluOpType.mult
    ADD = mybir.AluOpType.add

    ectr = [0]   # exp instruction counter (SE/DVE split)
    dctr = [0]   # drain instruction counter (DVE/SE alternation)

    with tile.TileContext(nc) as tc:
      with (tc.For_i(0, reps) if reps > 1 else contextlib.nullcontext()):
        with (
            tc.tile_pool(name="w", bufs=1) as wpool,
            tc.tile_pool(name="big", bufs=32) as big,
            tc.tile_pool(name="vp", bufs=NKT) as vpool,
            tc.tile_pool(name="at", bufs=8) as atp,
            tc.tile_pool(name="dr", bufs=4) as dpool,
            tc.tile_pool(name="sm", bufs=4) as small,
        ):
            # ---- weights ----
            wq_s = wpool.tile([128, 8, G], b16, tag="wq")
            wk_s = wpool.tile([128, 8, G], b16, tag="wk")
            wv_s = wpool.tile([128, 8, G], b16, tag="wv")
            for t in range(8):
                nc.sync.dma_start(wq_s[:, t, :], wqT[t * 128:(t + 1) * 128, :])
                nc.sync.dma_start(wk_s[:, t, :], wkT[t * 128:(t + 1) * 128, :])
                nc.sync.dma_start(wv_s[:, t, :], wvT[t * 128:(t + 1) * 128, :])
            if stage >= 5:
                wo_a = big.tile([128, 2, D], b16, tag="big", name="wo_a")
                wo_b = big.tile([128, 2, D], b16, tag="big", name="wo_b")
                for t in range(2):
                    nc.sync.dma_start(wo_a[:, t, :],
                                      woT[t * 128:(t + 1) * 128, :])
                    nc.sync.dma_start(wo_b[:, t, :],
                                      woT[(2 + t) * 128:(3 + t) * 128, :])
                wo_v = [wo_a[:, 0, :], wo_a[:, 1, :],
                        wo_b[:, 0, :], wo_b[:, 1, :]]

            # ablation plumbing: tiny live reads that defeat DCE per stage
            sink_t = wpool.tile([128, 512], f32, tag="sink")
            sink_n = [0]

            def sink(ap):
                c = sink_n[0]
                sink_n[0] += 1
                while len(ap.shape) > 2:
                    ap = ap[:, 0]
                nc.vector.tensor_copy(sink_t[0:1, c:c + 1], ap[0:1, 0:1])

            def drain_copy(dst, src):
                """psum -> sbuf copy, alternating DVE / ScalarE."""
                d = dctr[0]
                dctr[0] += 1
                if d % 2 == 0:
                    nc.vector.tensor_copy(dst, src)
                else:
                    nc.scalar.copy(dst, src)

            # ---- X^T inputs ----
            xq_s, xk_s, xv_s = [], [], []
            for src, dst in ((xqT, xq_s), (xkT, xk_s), (xvT, xv_s)):
                for t in range(8):
                    xt = big.tile([128, s], b16, tag="big")
                    nc.sync.dma_start(xt[:], src[t * 128:(t + 1) * 128, :])
                    dst.append(xt)

            if stage == 1:
                for xt in xq_s + xk_s + xv_s:
                    sink(xt)
                for wt in (wq_s, wk_s, wv_s):
                    sink(wt)

            with tc.tile_pool(name="ps", bufs=1, space="PSUM") as psum:
                # ---- Q^T / K^T projections: [G, s], out-dim on partitions --
                def proj_T(w_s, x_s, out_tiles, o):
                    ot = big.tile([128, s], b16, tag="big")
                    for half in range(NH):
                        ps = psum.tile([128, HALF], f32, tag="qk", bufs=QKB)
                        cs = slice(half * HALF, (half + 1) * HALF)
                        for i in range(8):
                            nc.tensor.matmul(
                                ps[:],
                                lhsT=w_s[:, i, o * 128:(o + 1) * 128],
                                rhs=x_s[i][:, cs],
                                start=(i == 0), stop=(i == 7),
                            )
                        drain_copy(ot[:, cs], ps[:])
                    out_tiles.append(ot)

                qT_s, kT_s = [], []

                # ---- V projection: vp_s[kt] = [128 kpos, 8 heads, 65] ----
                vp_s = []

                def emit_vproj(r):
                    ps = psum.tile([128, HALF], f32, tag="qk", bufs=QKB)
                    for i in range(8):
                        nc.tensor.matmul(
                            ps[:],
                            lhsT=xv_s[i][:, r * 128:(r + 1) * 128],
                            rhs=wv_s[:, i, :],
                            start=(i == 0), stop=(i == 7),
                        )
                    # ones column at index 64: Z lands on psum partition 64
                    # (32-aligned, required by the GPSIMD ops downstream).
                    vt = vpool.tile([128, H_LOCAL, DK + 1], b16, tag="vp")
                    nc.gpsimd.memset(vt[:, :, DK:DK + 1], 1.0)
                    drain_copy(
                        vt[:, :, 0:DK],
                        ps[:].rearrange("p (h d) -> p h d", h=H_LOCAL),
                    )
                    if debug:
                        nc.sync.dma_start(
                            vd[r * 128:(r + 1) * 128, :],
                            vt[:].rearrange("p h d -> p (h d)"))
                    vp_s.append(vt)

                if stage >= 2:
                    for o in range(4):
                        proj_T(wq_s, xq_s, qT_s, o)
                    for o in range(4):
                        proj_T(wk_s, xk_s, kT_s, o)
                    for r in range(NKT):
                        emit_vproj(r)

                if stage == 2:
                    for t in qT_s + kT_s + vp_s:
                        sink(t)

                # ---- attention: per q-chunk qh, per head pair ti ----
                aT_s = ([big.tile([128, s], b16, tag="big", name=f"aT{i}")
                         for i in range(4)] if stage >= 4 and not pv_only
                        else None)

                def attn_pair(ti, qh):
                    qs = slice(qh * QW, (qh + 1) * QW)
                    pr_A = slice(0, 64)
                    pr_B = slice(64, 128)
                    hA, hB = 2 * ti, 2 * ti + 1
                    if stage >= 4:
                        pvA = psum.tile([65, QW], f32, tag="pv", bufs=2,
                                        name=f"pvA{ti}_{qh}")
                        pvB = psum.tile([65, QW], f32, tag="pv", bufs=2,
                                        name=f"pvB{ti}_{qh}")
                    at_prev = None
                    # 2 k-tiles per psum tile: [128, 2, QW] spans two banks,
                    # exp reads the full 2*QW free extent in one instruction.
                    for kt2 in range(0, NKT + 2, 2):
                        if kt2 < NKT:
                            psA = psum.tile([128, 2, QW], f32, tag="qk",
                                            bufs=QKB)
                            psB = psum.tile([128, 2, QW], f32, tag="qk",
                                            bufs=QKB)
                            for j in range(2):
                                ks = slice((kt2 + j) * 128,
                                           (kt2 + j + 1) * 128)
                                nc.tensor.matmul(
                                    psA[:, j, :], lhsT=kT_s[ti][pr_A, ks],
                                    rhs=qT_s[ti][pr_A, qs],
                                    start=True, stop=True)
                                nc.tensor.matmul(
                                    psB[:, j, :], lhsT=kT_s[ti][pr_B, ks],
                                    rhs=qT_s[ti][pr_B, qs],
                                    start=True, stop=True)
                            at_c = []
                            for ps in (psA, psB):
                                e = ectr[0]
                                ectr[0] += 1
                                use_se = (((e * se_num) % 16) < se_num
                                          or not dve_exp)
                                if use_se:
                                    t = atp.tile([128, 2, QW], b16,
                                                 tag="ase", bufs=3)
                                    nc.scalar.activation(t[:], ps[:],
                                                         func=Exp)
                                else:
                                    t = atp.tile([128, 2, QW], i16,
                                                 tag="adv", bufs=3)
                                    nc.vector.tensor_scalar(
                                        t[:], ps[:], SCH_A, SCH_B,
                                        MULT, ADD)
                                if stage == 3:
                                    sink(t)
                                at_c.append((t, use_se))
                            at_cur = None if stage == 3 else at_c
                        if stage >= 4 and kt2 > 0 and at_prev is not None:
                            for hi_, (pv, hh) in enumerate(((pvA, hA),
                                                            (pvB, hB))):
                                tl, se_f = at_prev[hi_]
                                for j2 in range(2):
                                    j = kt2 - 2 + j2
                                    apj = tl[:, j2, :]
                                    nc.tensor.matmul(
                                        pv[:],
                                        lhsT=vp_s[j][:, hh, :],
                                        rhs=(apj if se_f
                                             else apj.bitcast(b16)),
                                        start=(j == 0),
                                        stop=(j == NKT - 1),
                                        skip_group_check=True)
                        if kt2 < NKT:
                            at_prev = at_cur
                    if stage < 4:
                        return
                    # ---- drain, on-chip Z: recip -> broadcast -> scale ----
                    otA = dpool.tile([65, QW], f32, tag="ot", bufs=3)
                    nc.vector.tensor_copy(otA[:], pvA[:])
                    otB = dpool.tile([65, QW], f32, tag="ot", bufs=3)
                    nc.scalar.copy(otB[:], pvB[:])
                    if pv_only:    # PV only: skip the normalize
                        sink(otA)
                        sink(otB)
                        return
                    # DVE reciprocal may cross base partitions (64 -> 0);
                    # HW partition_broadcast only reads partition 0; gpsimd
                    # tensor ops need all APs base-aligned, so the B head
                    # (output at base 64) normalizes on DVE instead.
                    rzA = dpool.tile([1, QW], f32, tag="rz", bufs=2)
                    nc.vector.reciprocal(rzA[0:1, :], otA[64:65, :])
                    rzB = dpool.tile([1, QW], f32, tag="rz", bufs=2)
                    nc.vector.reciprocal(rzB[0:1, :], otB[64:65, :])
                    zbA = dpool.tile([64, QW], f32, tag="zb", bufs=3)
                    nc.gpsimd.partition_broadcast(zbA[:], rzA[0:1, :])
                    zbB = dpool.tile([64, QW], f32, tag="zb", bufs=3)
                    nc.gpsimd.partition_broadcast(zbB[:], rzB[0:1, :])
                    nc.gpsimd.tensor_mul(aT_s[ti][pr_A, qs], otA[0:64, :],
                                         zbA[:])
                    nc.vector.tensor_mul(aT_s[ti][pr_B, qs], otB[0:64, :],
                                         zbB[:])

                # ---- output projection for one q-chunk ----
                def emit_oproj(qh):
                    qs = slice(qh * QW, (qh + 1) * QW)
                    for o in range(8):
                        ps = psum.tile([128, QW], f32, tag="fo", bufs=2)
                        for i in range(4):
                            nc.tensor.matmul(
                                ps[:],
                                lhsT=wo_v[i][:, o * 128:(o + 1) * 128],
                                rhs=aT_s[i][:, qs],
                                start=(i == 0), stop=(i == 3),
                            )
                        fo = small.tile([128, QW], f32, tag="fout",
                                        bufs=3)
                        drain_copy(fo[:], ps[:])
                        nc.sync.dma_start(outT[o * 128:(o + 1) * 128, qs],
                                          fo[:])

                if stage >= 3:
                    for qh in range(NQH):
                        for ti in range(4):
                            attn_pair(ti, qh)
                        if stage >= 5:
                            emit_oproj(qh)

                if stage == 4 and not pv_only:
                    for t in aT_s:
                        sink(t)
                if debug and stage >= 3:
                    for o in range(4):
                        nc.sync.dma_start(qTd[o * 128:(o + 1) * 128, :],
                                          qT_s[o][:])
                        nc.sync.dma_start(kTd[o * 128:(o + 1) * 128, :],
                                          kT_s[o][:])
                if debug and stage >= 4:
                    for i in range(4):
                        nc.sync.dma_start(aTd[i * 128:(i + 1) * 128, :],
                                          aT_s[i][:])

            if stage < 5:
                fo = small.tile([128, 512], f32, tag="fout", bufs=3)
                nc.vector.tensor_copy(fo[:], sink_t[:])
                nc.sync.dma_start(outT[0:128, 0:512], fo[:])

    nc.compile()
    return nc


def _host_prep(Q_in, K_in, V_in, W_q, W_k, W_v, W_o, s=S, reps=1,
               dve_exp=True, qk512=True, stage=5, se_num=9):
    """Build per-core input maps (host-side shard + transpose + bf16 cast)."""
    in_maps = []
    scale = 1.0 / np.sqrt(np.float32(DK))
    for c in range(N_CORES):
        b, g = divmod(c, 2)
        gs = slice(g * G, (g + 1) * G)
        m = {
            "xqT": np.ascontiguousarray(Q_in[b].T).astype(bf16),
            "xkT": np.ascontiguousarray(K_in[b].T).astype(bf16),
            "xvT": np.ascontiguousarray(V_in[b].T).astype(bf16),
            "wqT": np.ascontiguousarray((W_q[gs, :] * scale).T).astype(bf16),
            "wkT": np.ascontiguousarray(W_k[gs, :].T).astype(bf16),
            "wvT": np.ascontiguousarray(W_v[gs, :].T).astype(bf16),
            "woT": np.ascontiguousarray(W_o[:, gs].T).astype(bf16),
            "vtag": np.zeros((stage + 8 * reps + (64 if dve_exp else 0)
                              + (500 if qk512 else 0) + 7 * se_num,
                              _BUILD_VERSION), np.float32),
        }
        in_maps.append(m)
    return in_maps


def kernel(Q_in, K_in, V_in, W_q, W_k, W_v, W_o):
    from concourse.bass_utils import run_bass_kernel_spmd

    if "nc" not in _BUILD_CACHE:
        _BUILD_CACHE["nc"] = _build(qk512=True)
    nc = _BUILD_CACHE["nc"]

    in_maps = _host_prep(np.asarray(Q_in, np.float32), np.asarray(K_in, np.float32),
                         np.asarray(V_in, np.float32), np.asarray(W_q, np.float32),
                         np.asarray(W_k, np.float32), np.asarray(W_v, np.float32),
                         np.asarray(W_o, np.float32), qk512=True)
    res = run_bass_kernel_spmd(nc, in_maps, core_ids=list(range(N_CORES)))

    out = np.empty((B, S, D), np.float32)
    for b in range(B):
        acc = res.results[2 * b]["outT"] + res.results[2 * b + 1]["outT"]
        out[b] = acc.T
    return out
